# revision 3
# baseline (speedup 1.0000x reference)
"""DREAMReconstructor Trainium2 kernel.

Strategy: data-parallel over batch across 8 NeuronCores (8 rows/core).
Per core, a fully-unrolled 256-step recurrence where every matmul keeps
batch (M=8) as the PE stationary-column dim:

  - one combined weight matrix W_all = [W_rec; W_pred; W_dec] streamed as
    the moving operand, 4-way column-tiled across PE quadrants so the four
    K-chunks' waves run concurrently (out rows 32j hold h@W_rec (pre),
    h@W_pred (xhat), h@W_dec (recon)),
  - x@W_in.T precomputed for all t (xin), accumulated into the pre psum
    via identity-rhs transpose matmuls (no extra vector add),
  - -x_t accumulated into the xhat psum the same way, so the error
    reduction is a single ACT Square with accum_out,
  - low-rank fast weights A [(b,r)=128, h=512] live in SBUF; readout is a
    masked-selector matmul accumulated straight into pre; the Hebbian
    outer-product update is one K=8 matmul into psum plus a single
    fused scalar_tensor_tensor decay+add,
  - the leaky h update h' = tanh(pre)*iv + h*(1-iv) is computed by the PE
    via a stacked [40,512] stationary against a per-step diagonal matrix
    built with one fused vector op.
"""
import sys
import numpy as np

sys.path.insert(0, "/opt/trn_rl_repo")

import concourse.bass as bass
import concourse.tile as tile
from concourse import bacc, mybir
from concourse.bass_utils import run_bass_kernel_spmd

F32 = mybir.dt.float32

B, T, D, H, R = 64, 256, 256, 512, 16
NCORES = 8
BL = B // NCORES  # 8 rows per core

FORGET = 0.005
BASE_PLAST = 0.5
BASE_THRESH = 0.3
SURP_TEMP = 0.05
ERR_SMOOTH = 0.05
LTC_TAU = 5.0
LTC_SCALE = 5.0
KAPPA = 0.5
SLEEP_RATE = 0.01
MIN_SURP = 0.15

_CACHE = {}


def _host_constants(W_in, W_rec, W_pred, B_proj, W_dec, b_dec):
    c = {}
    W_all = np.concatenate([W_rec, W_pred, W_dec], axis=0)          # [1024, 512]
    c["W_allT"] = np.ascontiguousarray(W_all.T.reshape(4, 128, 1024, order="F")
                                       if False else
                                       W_all.T.reshape(512, 1024).reshape(4, 128, 1024))
    c["W_inT"] = np.ascontiguousarray(W_in.T.reshape(2, 128, H))     # [256,512]->[2,128,512]
    c["BprojT"] = np.ascontiguousarray(B_proj.T.reshape(2, 128, R))  # [256,16]->[2,128,16]
    c["I128"] = np.eye(128, dtype=np.float32)
    c["E_diag8"] = np.eye(8, dtype=np.float32)
    e = np.zeros((128, 8), np.float32)
    for p in range(128):
        e[p, p // 16] = KAPPA                                        # kappa folded into readout
    c["E_half"] = e
    c["E_bT"] = np.ascontiguousarray((e.T != 0).astype(np.float32))  # [8,128] 0/1
    es = np.zeros((40, 8), np.float32)
    eo = np.zeros((40, 8), np.float32)
    for b in range(8):
        es[b, b] = -1.0          # h_b side: om = 0.8 - surp
        eo[b, b] = 0.8
        es[32 + b, b] = 1.0      # th side: iv = 0.2 + surp
        eo[32 + b, b] = 0.2
    c["Esign40"] = es
    c["Eoff40"] = eo
    return c


def _build(nsteps):
    nc = bacc.Bacc("TRN2", target_bir_lowering=False, debug=False, num_devices=NCORES)

    d_xT = nc.dram_tensor("xT", [128, 2, T, BL], F32, kind="ExternalInput")
    d_WallT = nc.dram_tensor("W_allT", [4, 128, 1024], F32, kind="ExternalInput")
    d_WinT = nc.dram_tensor("W_inT", [2, 128, H], F32, kind="ExternalInput")
    d_BpT = nc.dram_tensor("BprojT", [2, 128, R], F32, kind="ExternalInput")
    d_I = nc.dram_tensor("I128", [128, 128], F32, kind="ExternalInput")
    d_nxT = nc.dram_tensor("nxT", [128, 2, T, BL], F32, kind="ExternalInput")
    d_Eh = nc.dram_tensor("E_half", [128, 8], F32, kind="ExternalInput")
    d_Ed8 = nc.dram_tensor("E_diag8", [8, 8], F32, kind="ExternalInput")
    d_EbT = nc.dram_tensor("E_bT", [8, 128], F32, kind="ExternalInput")
    d_Es = nc.dram_tensor("Esign40", [40, 8], F32, kind="ExternalInput")
    d_Eo = nc.dram_tensor("Eoff40", [40, 8], F32, kind="ExternalInput")
    d_y = nc.dram_tensor("y", [BL, T, D], F32, kind="ExternalOutput")

    AL = mybir.AluOpType
    AF = mybir.ActivationFunctionType

    with tile.TileContext(nc) as tc:
        with tc.tile_pool(name="persist", bufs=1) as P:
            # persistent tiles
            WallT = [P.tile([128, 1024], F32, tag=f"WallT{i}", name=f"WallT{i}") for i in range(4)]
            for kc in range(4):
                nc.sync.dma_start(WallT[kc][:], d_WallT[kc])
            I128 = P.tile([128, 128], F32); nc.sync.dma_start(I128[:], d_I[:])
            E_half = P.tile([128, 8], F32); nc.sync.dma_start(E_half[:], d_Eh[:])
            E_d8 = P.tile([8, 8], F32); nc.sync.dma_start(E_d8[:], d_Ed8[:])
            E_bT = P.tile([8, 128], F32); nc.sync.dma_start(E_bT[:], d_EbT[:])
            Es40 = P.tile([40, 8], F32); nc.sync.dma_start(Es40[:], d_Es[:])
            Eo40 = P.tile([40, 8], F32); nc.sync.dma_start(Eo40[:], d_Eo[:])

            xinT = P.tile([128, 4 * T * BL], F32)
            xinT_v = xinT[:].rearrange("p (hc t b) -> p hc t b", hc=4, t=T, b=BL)
            K_all = P.tile([128, T], F32)
            A_sb = P.tile([128, H], F32)
            hT = P.tile([128, 32], F32)
            hh = P.tile([40, H], F32)
            surp40 = P.tile([40, 1], F32)
            nE20 = P.tile([8, 1], F32)
            errsum = P.tile([128, 1], F32)
            lt8 = P.tile([8, 1], F32)
            sa8 = P.tile([8, 1], F32)
            sleep8 = P.tile([8, 1], F32)
            sp2 = P.tile([8, 1], F32)
            r8 = P.tile([8, 1], F32)
            tmp8 = P.tile([8, 1], F32)
            sa_sb = P.tile([128, 1], F32)
            kdot = P.tile([8, T], F32)
            c_nthr = P.tile([8, 1], F32)

            nc.vector.memset(A_sb[:], 0.0)
            nc.vector.memset(hT[:], 0.0)
            nc.vector.memset(hh[:], 0.0)
            nc.vector.memset(surp40[:], 0.0)
            nc.vector.memset(nE20[:], 0.0)
            nc.vector.memset(c_nthr[:], float(-BASE_THRESH))
            nc.vector.memset(sa_sb[:], 1.0)
            nc.vector.memset(sp2[:], 0.0)
            nc.vector.memset(kdot[:], 0.0)

            # ---------------- precompute: xin = x @ W_in.T, k = x @ B_proj.T
            with tc.tile_pool(name="pre_sb", bufs=3) as PS, \
                 tc.tile_pool(name="pre_ps", bufs=2, space="PSUM") as PP:
                WinT = [PS.tile([128, H], F32, tag=f"WinT{i}", name=f"WinT{i}") for i in range(2)]
                BpT = [PS.tile([128, R], F32, tag=f"BpT{i}", name=f"BpT{i}") for i in range(2)]
                for dc in range(2):
                    nc.sync.dma_start(WinT[dc][:], d_WinT[dc])
                    nc.sync.dma_start(BpT[dc][:], d_BpT[dc])
                TB = 64  # t-block
                for tb in range(T // TB):
                    rhs = [PS.tile([128, TB * BL], F32, tag=f"xrhs{i}", name=f"xrhs{i}") for i in range(2)]
                    for dc in range(2):
                        nc.sync.dma_start(
                            rhs[dc][:],
                            d_xT[:, dc, tb * TB:(tb + 1) * TB, :])
                    for hc in range(4):
                        ps = PP.tile([128, TB * BL], F32, tag="xps")
                        nc.tensor.matmul(ps[:], (WinT[0][:, 128 * hc:128 * (hc + 1)]),
                                         (rhs[0][:]), start=True, stop=False)
                        nc.tensor.matmul(ps[:], (WinT[1][:, 128 * hc:128 * (hc + 1)]),
                                         (rhs[1][:]), start=False, stop=True)
                        eng = nc.vector if hc % 2 == 0 else nc.scalar
                        if hc % 2 == 0:
                            nc.vector.tensor_copy(
                                xinT_v[:, hc, tb * TB:(tb + 1) * TB, :], ps[:])
                        else:
                            nc.scalar.copy(
                                xinT_v[:, hc, tb * TB:(tb + 1) * TB, :], ps[:])
                    psk = PP.tile([128, TB * BL], F32, tag="xps")
                    nc.tensor.matmul(psk[0:16, :], (BpT[0][:]), (rhs[0][:]),
                                     start=True, stop=False)
                    nc.tensor.matmul(psk[0:16, :], (BpT[1][:]), (rhs[1][:]),
                                     start=False, stop=True)
                    kb = PS.tile([16, TB * BL], F32, tag="kb")
                    nc.vector.tensor_copy(kb[:], psk[0:16, :])
                    kb_v = kb[:].rearrange("r (t b) -> r t b", t=TB, b=BL)
                    for b in range(8):
                        nc.sync.dma_start(
                            K_all[16 * b:16 * (b + 1), tb * TB:(tb + 1) * TB],
                            kb_v[:, :, b])

                # kdot[b, t] = 0.5 * <k_t, k_{t-1}> (E_half carries the 0.5)
                prodk = PS.tile([128, T - 1], F32, tag="prodk")
                nc.vector.tensor_tensor(prodk[:], K_all[:, 1:T], K_all[:, 0:T - 1],
                                        AL.mult)
                psd = PP.tile([128, T - 1], F32, tag="xps")
                nc.tensor.matmul(psd[0:8, :], E_half[:], prodk[:],
                                 start=True, stop=True)
                nc.vector.tensor_copy(kdot[:, 1:T], psd[0:8, :])

            # ---------------- recurrent loop
            with tc.tile_pool(name="pm", bufs=2, space="PSUM") as PM, \
                 tc.tile_pool(name="pa", bufs=2, space="PSUM") as PA, \
                 tc.tile_pool(name="ph", bufs=2, space="PSUM") as PH, \
                 tc.tile_pool(name="pb", bufs=2, space="PSUM") as PB, \
                 tc.tile_pool(name="step", bufs=2) as SP, \
                 tc.tile_pool(name="xring", bufs=8) as XR:
                # j -> W_all column block: g0=xhat(pred), g1=pre-lo(rec0),
                # g2=pre-hi(rec1), g3=recon(dec)
                JBLK = {0: 2, 1: 0, 2: 1, 3: 3}

                def emit_gmm(tt, pm_t):
                    """Start pre regions of step tt with the decayed old-A
                    readout (reads A_sb pre-update and last sa_sb)."""
                    ksg = SP.tile([128, 8], F32, tag="kselg", name=f"ksg{tt}")
                    nc.vector.tensor_scalar(ksg[:], E_half[:],
                                            K_all[:, tt:tt + 1], sa_sb[:],
                                            AL.mult, AL.mult)
                    nc.tensor.matmul(pm_t[32:40, 0:256], ksg[:], A_sb[:, 0:256],
                                     start=True, stop=False, tile_position=(0, 32))
                    nc.tensor.matmul(pm_t[64:72, 0:256], ksg[:], A_sb[:, 256:512],
                                     start=True, stop=False, tile_position=(0, 64))

                pm_cur = PM.tile([128, 512], F32, tag="pm", name="pm_pro")
                emit_gmm(0, pm_cur)

                for t in range(nsteps):
                    pm = pm_cur
                    xt = XR.tile([128, 2, BL], F32, tag="xt")
                    nc.sync.dma_start(xt[:], d_nxT[:, :, t, :])

                    pa = PA.tile([128, 512], F32, tag="pa")
                    ph = PH.tile([128, 64], F32, tag="ph")
                    pb = PB.tile([8, 512], F32, tag="pb")

                    # c-term: fast_t += c8 * h_t, c8 = 0.5*sp2_{t-1}*kdot[:,t]
                    # (reads sp2 and hh[0:8] BEFORE this step overwrites them)
                    c8 = SP.tile([8, 1], F32, tag="c8")
                    nc.vector.scalar_tensor_tensor(c8[:], kdot[:, t:t + 1], 0.5,
                                                   sp2[:], AL.mult, AL.mult)
                    Dc = SP.tile([8, 8], F32, tag="dc")
                    nc.vector.tensor_scalar(Dc[:], E_d8[:], c8[:], None, AL.mult)
                    nc.tensor.matmul(pm[32:40, 0:256], Dc[:], hh[0:8, 0:256],
                                     start=False, stop=False, tile_position=(0, 32))
                    nc.tensor.matmul(pm[64:72, 0:256], Dc[:], hh[0:8, 256:512],
                                     start=False, stop=False, tile_position=(0, 64))

                    # -x_t starts xhat (transpose-mode at psum partition 0)
                    for dc in range(2):
                        nc.tensor.matmul(
                            pm[0:8, 128 * dc:128 * (dc + 1)],
                            xt[:, dc, :], I128[:], is_transpose=True,
                            start=(dc == 0), stop=False, tile_position=(0, 0))
                    # xin into the pre regions
                    for cx in range(4):
                        j = 1 + cx // 2
                        nc.tensor.matmul(
                            pm[32 * j:32 * j + 8, 128 * (cx % 2):128 * (cx % 2 + 1)],
                            xinT_v[:, cx, t, :], I128[:],
                            start=False, stop=False, tile_position=(0, 32 * j))

                    # big waves: 4 K-chunks x 4 concurrent col groups (N=256);
                    # xhat group first, wave kc==3 closes every region
                    for kc in range(4):
                        for j in (0, 1, 2, 3):
                            blk = JBLK[j]
                            nc.tensor.matmul(
                                pm[32 * j:32 * j + 8, 0:256],
                                hT[:, 8 * kc:8 * (kc + 1)],
                                WallT[kc][:, 256 * blk:256 * (blk + 1)],
                                start=(kc == 0 and j == 3),
                                stop=(kc == 3),
                                tile_position=(0, 32 * j))

                    # error chain
                    sqd = SP.tile([8, 256], F32, tag="sqd")
                    nc.scalar.activation(sqd[:], pm[0:8, 0:256], AF.Square,
                                         accum_out=errsum[0:8, :])
                    nc.scalar.activation(surp40[0:8, :], errsum[0:8, :], AF.Sigmoid,
                                         bias=nE20[:], scale=float(1.0 / (256 * SURP_TEMP)))
                    nc.scalar.activation(surp40[32:40, :], errsum[0:8, :], AF.Sigmoid,
                                         bias=nE20[:], scale=float(1.0 / (256 * SURP_TEMP)))
                    D40 = SP.tile([40, 8], F32, tag="d40")
                    nc.vector.scalar_tensor_tensor(D40[:], Es40[:], surp40[:],
                                                   Eo40[:], AL.mult, AL.add)
                    # rest of the small chain (off critical)
                    nc.vector.tensor_scalar(tmp8[:], errsum[0:8, :],
                                            float(-1.0 / 256.0), None, AL.mult)
                    nc.vector.scalar_tensor_tensor(nE20[:], nE20[:],
                                                   float(1.0 - ERR_SMOOTH),
                                                   tmp8[:], AL.mult, AL.add)
                    nc.scalar.activation(r8[:], surp40[0:8, :], AF.Relu,
                                         bias=c_nthr[:])
                    nc.vector.tensor_scalar(lt8[:], surp40[0:8, :], float(MIN_SURP),
                                            None, AL.is_lt)
                    nc.vector.tensor_scalar(sleep8[:], lt8[:], float(-SLEEP_RATE),
                                            1.0, AL.mult, AL.add)
                    nc.vector.tensor_scalar(sa8[:], lt8[:],
                                            float(-SLEEP_RATE * (1.0 - FORGET)),
                                            float(1.0 - FORGET), AL.mult, AL.add)
                    nc.vector.tensor_tensor(sp2[:], r8[:], sleep8[:], AL.mult)
                    nc.tensor.matmul(ph[0:128, 32:33], E_bT[:], sa8[:],
                                     start=True, stop=True)
                    nc.vector.tensor_copy(sa_sb[:], ph[:, 32:33])

                    # tanh into hh rows 32:40 (pre at g1/g2)
                    nc.scalar.activation(hh[32:40, 0:256], pm[32:40, 0:256], AF.Tanh)
                    nc.scalar.activation(hh[32:40, 256:512], pm[64:72, 0:256], AF.Tanh)

                    # h_newT = hh[0:40].T @ D40 -> ph cols 0:32
                    for kc in range(4):
                        nc.tensor.matmul(ph[:, 8 * kc:8 * (kc + 1)],
                                         hh[:, 128 * kc:128 * (kc + 1)], D40[:],
                                         start=True, stop=True)
                    hT_n = hT
                    nc.vector.tensor_copy(hT_n[:], ph[:, 0:32])

                    # back-transpose h_new -> pb rows 0:8 -> hh rows 0:8
                    for hc in range(4):
                        nc.tensor.matmul(pb[0:8, 128 * hc:128 * (hc + 1)],
                                         hT_n[:, 8 * hc:8 * (hc + 1)], I128[:],
                                         is_transpose=True, start=True, stop=True)
                    nc.scalar.copy(hh[0:8, 0:256], pb[0:8, 0:256])
                    nc.vector.tensor_copy(hh[0:8, 256:512], pb[0:8, 256:512])

                    # recon_{t-1} out (bounce via sbuf)
                    if t > 0:
                        rec_sb = SP.tile([8, 256], F32, tag="rec", bufs=3)
                        if t % 2 == 0:
                            nc.scalar.copy(rec_sb[:], pm[96:104, 0:256])
                        else:
                            nc.vector.tensor_copy(rec_sb[:], pm[96:104, 0:256])
                        nc.sync.dma_start(d_y[:, t - 1, :], rec_sb[:])

                    # next step's pre starter must read A_sb BEFORE this step's
                    # A update
                    if t < nsteps - 1:
                        pm_cur = PM.tile([128, 512], F32, tag="pm",
                                         name=f"pm{t + 1}")
                        emit_gmm(t + 1, pm_cur)

                    # A outer product + decay update (2 steps of slack)
                    K_sel = SP.tile([128, 8], F32, tag="ksel")
                    nc.vector.tensor_scalar(K_sel[:], E_half[:], K_all[:, t:t + 1],
                                            None, AL.mult)
                    nc.tensor.matmul(pm[0:8, 256:384], K_sel[:], I128[:],
                                     is_transpose=True, start=True, stop=True)
                    K_blk = SP.tile([8, 128], F32, tag="kblk")
                    nc.vector.tensor_scalar(K_blk[:], pm[0:8, 256:384], sp2[:],
                                            None, AL.mult)
                    nc.tensor.matmul(pa[:, 0:512], K_blk[:], hh[0:8, 0:512],
                                     start=True, stop=True)
                    nc.vector.scalar_tensor_tensor(A_sb[:], A_sb[:], sa_sb[:],
                                                   pa[:], AL.mult, AL.add)

                # final recon for t = nsteps-1
                pmf = PM.tile([128, 512], F32, tag="pm")
                for kc in range(4):
                    nc.tensor.matmul(pmf[96:104, 0:256],
                                     hT[:, 8 * kc:8 * (kc + 1)],
                                     WallT[kc][:, 768:1024],
                                     start=(kc == 0), stop=(kc == 3),
                                     tile_position=(0, 96))
                rec_f = SP.tile([8, 256], F32, tag="rec", bufs=3)
                nc.vector.tensor_copy(rec_f[:], pmf[96:104, 0:256])
                nc.sync.dma_start(d_y[:, nsteps - 1, :], rec_f[:])

    nc.finalize()
    return nc


def _make_runner(nc):
    """Persistent jitted SPMD executor (mirrors bass2jax.run_bass_via_pjrt,
    but reusable across calls so the NEFF stays loaded on the devices)."""
    import jax
    from jax.experimental.shard_map import shard_map
    from jax.sharding import Mesh, PartitionSpec
    from concourse import bass2jax
    from concourse import mybir as mb

    bass2jax.install_neuronx_cc_hook()

    partition_name = (nc.partition_id_tensor.name
                      if nc.partition_id_tensor else None)
    in_names, out_names, out_avals, zero_outs = [], [], [], []
    for alloc in nc.m.functions[0].allocations:
        if not isinstance(alloc, mb.MemoryLocationSet):
            continue
        name = alloc.memorylocations[0].name
        if alloc.kind == "ExternalInput":
            if name != partition_name:
                in_names.append(name)
        elif alloc.kind == "ExternalOutput":
            out_names.append(name)
            shape = tuple(alloc.tensor_shape)
            dtype = mb.dt.np(alloc.dtype)
            out_avals.append(jax.core.ShapedArray(shape, dtype))
            zero_outs.append(np.zeros(shape, dtype))
    n_params = len(in_names)
    n_outs = len(out_avals)
    all_in_names = list(in_names) + list(out_names)
    if partition_name is not None:
        all_in_names.append(partition_name)

    def _body(*args):
        operands = list(args)
        if partition_name is not None:
            operands.append(bass2jax.partition_id_tensor())
        outs = bass2jax._bass_exec_p.bind(
            *operands,
            out_avals=tuple(out_avals),
            in_names=tuple(all_in_names),
            out_names=tuple(out_names),
            lowering_input_output_aliases=(),
            sim_require_finite=True,
            sim_require_nnan=True,
            nc=nc,
        )
        return tuple(outs)

    devices = jax.devices()[:NCORES]
    mesh = Mesh(np.asarray(devices), ("core",))
    in_specs = (PartitionSpec("core"),) * (n_params + n_outs)
    out_specs = (PartitionSpec("core"),) * len(out_names)
    donate = tuple(range(n_params, n_params + n_outs))
    sharded = jax.jit(
        shard_map(_body, mesh=mesh, in_specs=in_specs, out_specs=out_specs,
                  check_rep=False),
        donate_argnums=donate, keep_unused=True)

    def run(in_maps):
        concat_in = [
            np.concatenate([np.asarray(in_maps[c][n]) for c in range(NCORES)],
                           axis=0)
            for n in in_names
        ]
        concat_zeros = [
            np.zeros((NCORES * z.shape[0], *z.shape[1:]), z.dtype)
            for z in zero_outs
        ]
        out_arrs = sharded(*concat_in, *concat_zeros)
        return [
            {n: np.asarray(out_arrs[i]).reshape(NCORES, *out_avals[i].shape)[c]
             for i, n in enumerate(out_names)}
            for c in range(NCORES)
        ]

    return run


def kernel(x, W_in, W_rec, W_pred, B_proj, W_dec, b_dec, _nsteps=T, _trace=False):
    x = np.asarray(x, np.float32)
    consts = _host_constants(np.asarray(W_in, np.float32),
                             np.asarray(W_rec, np.float32),
                             np.asarray(W_pred, np.float32),
                             np.asarray(B_proj, np.float32),
                             np.asarray(W_dec, np.float32),
                             np.asarray(b_dec, np.float32))
    key = _nsteps
    if key not in _CACHE:
        nc = _build(_nsteps)
        _CACHE[key] = (nc, _make_runner(nc))
    nc, run = _CACHE[key]

    in_maps = []
    for c in range(NCORES):
        xs = x[c * BL:(c + 1) * BL]                      # [8, T, D]
        xT = np.ascontiguousarray(
            xs.transpose(2, 1, 0).reshape(2, 128, T, BL).transpose(1, 0, 2, 3))
        m = dict(consts)
        m["xT"] = xT
        m["nxT"] = np.ascontiguousarray(-xT)
        in_maps.append(m)

    if _trace:
        import os
        import types
        import concourse.bass_utils as bu
        if "antenv.axon_hooks" not in sys.modules:
            mod = types.ModuleType("antenv.axon_hooks")
            mod._hook = None
            mod.set_axon_ntff_profile_hook = lambda h: setattr(mod, "_hook", h)
            mod.get_axon_ntff_profile_hook = lambda: mod._hook
            sys.modules["antenv.axon_hooks"] = mod
            from trn_agent_boot.trn_boot import _ntff_profile_via_ctypes
            mod._hook = _ntff_profile_via_ctypes("/opt/axon/libaxon_pjrt.so")
        bu.upload_artifacts = lambda tmpdir: "local://" + tmpdir
        tmpdir = os.environ.get("TRACE_DIR") or None
        res = run_bass_kernel_spmd(nc, in_maps, core_ids=list(range(NCORES)),
                                   trace=True, tmpdir=tmpdir)
        kernel.last_exec_time_ns = res.exec_time_ns
        kernel.last_results = res
        results = res.results
        y = np.concatenate([results[c]["y"] for c in range(NCORES)], axis=0)
        bd = np.asarray(b_dec, np.float32)
        if np.any(bd):
            y = y + bd[None, None, :]
        return y

    results = run(in_maps)
    y = np.concatenate([results[c]["y"] for c in range(NCORES)], axis=0)
    bd = np.asarray(b_dec, np.float32)
    if np.any(bd):
        y = y + bd[None, None, :]
    return y

